# revision 23
# baseline (speedup 1.0000x reference)
"""Pairwise cosine-distance matrix kernel for Trainium2 (Bass/Tile, 8 cores).

Problem: mapping [8192, 512] fp32 -> out[i,j] = 1 - <x_i, x_j> / (|x_i||x_j|),
full [8192, 8192] fp32 output.

Strategy (SPMD over 8 NeuronCores, symmetric-triangle partitioning):
  - Only the 136 distinct [512, 512] blocks of the 16x16 block grid need
    device compute (output is symmetric). Circulant assignment: core c owns
    row-blocks c (9 column blocks) and c+8 (8 column blocks) = 17 blocks,
    SPMD-uniform thanks to a host-side rotation of the column tiles by c.
  - Rows are L2-normalized ON HOST, scaled by S=16, and quantized to
    fp8e4 (TRN E4M3). The host builds the exact tile-major SBUF image per
    core, so the device input DMA is 16 fully-linear [128, 2048 B] chunks.
  - Gram blocks run as fp8 DoubleRow matmuls (2 fp8 weights/cell, K=256
    per instruction): 136 MMs of [128x512] out, fp32 PSUM accumulate over
    2 k-pairs. ~2x PE throughput vs the fp16 variant.
  - Epilogue is a single scale op (psum * 1/32 -> 8*cos), alternating
    ACT/DVE, writing fp8e3 (E3M4, 4 mantissa bits) staging tiles -> output
    DMA is half of an fp16 variant. No (1-x) on device; the host computes
    1 - A/8 during upcast/assembly.
  - The diagonal (exactly 0 up to fp32 rounding, |ref| < 4e-6) is patched
    on host, which removes the dominant fp8 quantization error mode
    (||q_i||^2 != 1). Measured numpy-oracle rel err: 9.3e-3 vs 2e-2 gate.
"""

import json
import os
import sys
import types

import numpy as np

N = 8192
D = 512
N_CORES = 8
NB = 16                 # 512-wide row/col blocks
BS = N // NB            # 512
KC = D // 128           # 4 k-chunks of 128
MT = BS // 128          # 4 row-chunks of 128 per 512-row part
S_IN = 16.0             # host input scale before fp8e4 quantization
# psum = S_IN^2 * cos = 256*cos; stage = psum/32 = 8*cos (diag = 8 < e3m4 max)
EPI_SCALE = 1.0 / 32.0
OUT_SCALE = 8.0         # host divides by this and does 1 - x

# (weight_tile, moving-tile groups, out_name). 4-wide groups keep PSUM
# at 4+4 of 8 banks and output-DMA descriptors at 2 KB/partition (matching
# the 2 KB input descriptors — the SDMA engines round-robin rings at
# packet granularity, so mismatched descriptor sizes starve the small
# side). The lone [8] group for wt=0 runs LAST: its weight tile is
# already resident and its tiny stage shortens the final drain.
GRAM_PLAN = [
    (0, [[0, 1, 2, 3], [4, 5, 6, 7]], "outA"),
    (8, [[8, 9, 10, 11], [12, 13, 14, 15]], "outB"),
    (0, [[8]], "outA"),
]

LAST_EXEC_NS = None  # max-across-traced-cores HW time of the last profiled run

_cached = {}


def _install_ntff_hook():
    """bass_utils' trace path imports antenv.axon_hooks, which this image
    lacks; recreate it and register the ctypes NTFF hook (same thing the
    boot script would have done)."""
    if "antenv.axon_hooks" in sys.modules:
        return
    mod = types.ModuleType("antenv.axon_hooks")
    holder = [None]
    mod.set_axon_ntff_profile_hook = lambda h: holder.__setitem__(0, h)
    mod.get_axon_ntff_profile_hook = lambda: holder[0]
    sys.modules["antenv.axon_hooks"] = mod
    import antenv
    antenv.axon_hooks = mod
    try:
        from trn_agent_boot.trn_boot import _ntff_profile_via_ctypes
        mod.set_axon_ntff_profile_hook(
            _ntff_profile_via_ctypes("/opt/axon/libaxon_pjrt.so")
        )
    except Exception:
        pass


def _split_multiwait_bir(bir_json: bytes) -> bytes:
    """This container's walrus rejects instructions with >1 semaphore wait
    ("Too many sync wait commands"). Hoist extra waits onto standalone
    wait-only EventSemaphore instructions placed just before, on the same
    engine — identical stall semantics."""
    m = json.loads(bir_json)
    for f in m["functions"]:
        for bb in f.get("blocks", f.get("basicblocks", [])):
            new_insts = []
            for inst in bb["instructions"]:
                si = inst.get("sync_info")
                waits = si.get("on_wait") if si else None
                if waits and len(waits) > 1:
                    for j, w in enumerate(waits[:-1]):
                        new_insts.append({
                            "debug": inst.get("debug"),
                            "engine": inst["engine"],
                            "ins": [],
                            "name": f"{inst['name']}-hw{j}",
                            "opcode": "EventSemaphore",
                            "outs": [],
                            "sync_info": {"on_update": [], "on_wait": [w]},
                        })
                    si["on_wait"] = [waits[-1]]
                new_insts.append(inst)
            bb["instructions"] = new_insts
    return json.dumps(m).encode()


def _apply_patches():
    if _cached.get("patched"):
        return
    _cached["patched"] = True
    import concourse.bass2jax as bass2jax
    import concourse.bass_utils as bass_utils

    orig_compile = bass2jax.compile_bir_kernel

    def patched_compile(bir_json, tmpdir, neff_name="file.neff"):
        return orig_compile(_split_multiwait_bir(bir_json), tmpdir,
                            neff_name=neff_name)

    bass2jax.compile_bir_kernel = patched_compile
    # No S3 in this container; the trace path uploads artifacts for links only.
    bass_utils.upload_artifacts = lambda tmpdir: "local://" + tmpdir


def _build():
    key = "nc"
    if key in _cached:
        return _cached[key]
    _apply_patches()
    import concourse.bass as bass
    import concourse.tile as tile
    from concourse import mybir

    f32 = mybir.dt.float32
    f8i = mybir.dt.float8e4   # input/matmul dtype
    f8o = mybir.dt.float8e3   # output staging dtype (4 mantissa bits)
    Act = mybir.ActivationFunctionType
    Alu = mybir.AluOpType
    DR = mybir.MatmulPerfMode.DoubleRow

    nc = bass.Bass(trn_type="TRN2", target_bir_lowering=False, debug=False)
    # tile-major SBUF image: [partition, tile, k-chunk, col]
    xt_d = nc.dram_tensor("xt", [128, NB, KC, BS], f8i, kind="ExternalInput").ap()
    outA_d = nc.dram_tensor("outA", [BS, 9 * BS], f8o, kind="ExternalOutput").ap()
    outB_d = nc.dram_tensor("outB", [BS, 8 * BS], f8o, kind="ExternalOutput").ap()

    with tile.TileContext(nc) as tc:
        with (
            tc.tile_pool(name="xt", bufs=1) as xt_pool,
            tc.tile_pool(name="warm", bufs=1) as warm_pool,
            tc.tile_pool(name="stage", bufs=6) as stage_pool,
            tc.tile_pool(name="ps", bufs=8, space=bass.MemorySpace.PSUM) as ps,
        ):
            xt = xt_pool.tile([128, NB, KC, BS], f8i, name="xt")

            # PE warm-up: dummy DoubleRow matmuls over the (not yet loaded)
            # tile-15 region of xt — garbage values, results discarded, no
            # memset gate, so they issue the moment the PE sequencer boots
            # (~7.5 us) and the HAM clock-gate releases (K=8/8) right as
            # the first real matmul's data lands; the real stream then runs
            # entirely at 2.4 GHz. (Full-size matmuls: small ones don't
            # register enough PE-array activity to release the gate.) The
            # only ordering edge is WAR: tile 15's input DMA (issued ~17
            # us) waits for the last dummy (~11 us) — free.
            wps = ps.tile([128, BS], f32, tag="pg", name="warm_ps")
            for i in range(8):
                nc.tensor.matmul(wps[:], xt[:, 15, 0:2, 0:128],
                                 xt[:, 15, 0:2, :],
                                 start=True, stop=True, perf_mode=DR)

            # preload the ACT activation table at boot so the first real
            # epilogue op doesn't stall 1.3 us on ACT_TABLE_LOAD. Dedicated
            # scratch so nothing serializes behind it.
            tab = warm_pool.tile([1, 2], f32, name="tab")
            nc.gpsimd.memset(tab[:], 0.0)

            # input DMA: one [128, 2048 B] linear chunk per tile, in
            # consumption order. Uniform 2 KB descriptors keep the SDMA
            # packet round-robin fair vs the output DMAs; bigger bulk DMAs
            # were measured to starve everything else. Tiles 1 and 3 issue
            # on the otherwise-idle ACT ring: issue is ~0.63 us serialized
            # per ring and each completion sem adds ~2 us, so two rings
            # get the first four tiles ready ~1.5 us sooner.
            nc.sync.dma_start(out=xt[:, 0, :, :], in_=xt_d[:, 0, :, :])
            nc.scalar.dma_start(out=xt[:, 1, :, :], in_=xt_d[:, 1, :, :])
            nc.scalar.activation(tab[:, 1:2], tab[:, 0:1],
                                 Act.Identity, scale=EPI_SCALE)
            for t in range(2, NB):
                nc.sync.dma_start(out=xt[:, t, :, :], in_=xt_d[:, t, :, :])

            def w_ap(wt, kp, mt):
                return xt[:, wt, 2 * kp:2 * kp + 2, 128 * mt:128 * (mt + 1)]

            def m_ap(t, kp):
                return xt[:, t, 2 * kp:2 * kp + 2, :]

            epi_ctr = 0
            for wt, groups, out_name in GRAM_PLAN:
                out_d = outA_d if out_name == "outA" else outB_d
                base_t = 0 if out_name == "outA" else 8
                for g in groups:
                    for mt in range(MT):
                        # diagonal block (t == wt): only the upper
                        # mt-trapezoid (cols >= 128*mt) is computed; the
                        # host mirrors the rest. Saves ~1.3 us of PE time.
                        los = [128 * mt if t == wt else 0 for t in g]
                        psums = [ps.tile([128, BS - lo], f32, tag="pg",
                                         name=f"pg_{wt}_{t}_{mt}")
                                 for t, lo in zip(g, los)]
                        # MM order: narrow diagonal matmuls (shorter than a
                        # DoubleRow LDWEIGHTS) are interleaved after full
                        # ones so the next weight load hides; plain order
                        # otherwise.
                        if los[0]:
                            order = [(1, 0), (0, 0), (1, 1), (0, 1)]
                            order += [(j, kp) for j in range(2, len(g))
                                      for kp in range(2)]
                        else:
                            order = [(j, kp) for j in range(len(g))
                                     for kp in range(2)]
                        for j, kp in order:
                            nc.tensor.matmul(
                                psums[j][:], w_ap(wt, kp, mt),
                                xt[:, g[j], 2 * kp:2 * kp + 2, los[j]:BS],
                                start=(kp == 0), stop=(kp == 1),
                                perf_mode=DR)
                        nt = len(g)
                        stage = stage_pool.tile([128, nt * BS], f8o, tag="st",
                                                name=f"st_{wt}_{g[0]}_{mt}")
                        for j in range(nt):
                            lo = los[j]
                            ssl = slice(j * BS + lo, (j + 1) * BS)
                            if nt == 1:
                                # tail chunks: split one block across both
                                # engines to halve the final drain latency
                                h = BS // 2
                                nc.scalar.activation(stage[:, 0:h],
                                                     psums[j][:, 0:h],
                                                     Act.Identity,
                                                     scale=EPI_SCALE)
                                nc.vector.tensor_scalar(stage[:, h:BS],
                                                        psums[j][:, h:BS],
                                                        EPI_SCALE, None,
                                                        Alu.mult)
                            elif epi_ctr % 2 == 0:
                                nc.scalar.activation(stage[:, ssl], psums[j][:],
                                                     Act.Identity,
                                                     scale=EPI_SCALE)
                            else:
                                nc.vector.tensor_scalar(stage[:, ssl],
                                                        psums[j][:],
                                                        EPI_SCALE, None,
                                                        Alu.mult)
                            epi_ctr += 1
                        off = (g[0] - base_t) * BS
                        lo0 = los[0]
                        nc.sync.dma_start(
                            out=out_d[mt * 128:(mt + 1) * 128,
                                      off + lo0:off + nt * BS],
                            in_=stage[:, lo0:nt * BS])

    _cached[key] = nc
    return nc


def _prep_inputs(mapping: np.ndarray) -> list:
    import ml_dtypes
    x = np.ascontiguousarray(mapping, dtype=np.float32)
    norms = np.sqrt((x.astype(np.float64) ** 2).sum(axis=1))
    xh = (x / np.maximum(norms, 1e-30)[:, None]).astype(np.float32)
    q = (xh * S_IN).astype(ml_dtypes.float8_e4m3)   # [8192, 512] fp8
    # SBUF image for core 0: B[p, t, k, c] = q[512 t + c, 128 k + p]
    qT = np.ascontiguousarray(q.T)                  # [512, 8192]
    base = qT.reshape(KC, 128, NB, BS).transpose(1, 2, 0, 3)  # [p, t, k, c]
    in_maps = []
    for c in range(N_CORES):
        img = np.ascontiguousarray(np.roll(base, -c, axis=1))
        in_maps.append({"xt": img})
    return in_maps


def kernel(mapping: np.ndarray) -> np.ndarray:
    from concourse.bass_utils import run_bass_kernel_spmd

    assert mapping.shape == (N, D)
    in_maps = _prep_inputs(mapping)
    nc = _build()

    trace = bool(int(os.environ.get("BASSKNN_TRACE", "0")))
    if trace:
        _install_ntff_hook()
    res = run_bass_kernel_spmd(nc, in_maps, list(range(N_CORES)), trace=trace)
    global LAST_EXEC_NS
    if trace:
        LAST_EXEC_NS = res.exec_time_ns

    inv = np.float32(1.0 / OUT_SCALE)
    full = np.empty((N, N), np.float32)
    for c in range(N_CORES):
        A = np.asarray(res.results[c]["outA"]).astype(np.float32)
        B = np.asarray(res.results[c]["outB"]).astype(np.float32)
        A = 1.0 - A * inv
        B = 1.0 - B * inv
        for t in range(9):
            j = (c + t) % NB
            blk = A[:, t * BS:(t + 1) * BS]
            full[c * BS:(c + 1) * BS, j * BS:(j + 1) * BS] = blk
            if t:
                full[j * BS:(j + 1) * BS, c * BS:(c + 1) * BS] = blk.T
        i2 = c + 8
        for e in range(8):
            j = (i2 + e) % NB
            blk = B[:, e * BS:(e + 1) * BS]
            full[i2 * BS:(i2 + 1) * BS, j * BS:(j + 1) * BS] = blk
            if e:
                full[j * BS:(j + 1) * BS, i2 * BS:(i2 + 1) * BS] = blk.T
    # diagonal blocks: device computed only the upper mt-trapezoid;
    # mirror the strict lower triangle from the (valid) upper part
    tri = np.tril_indices(BS, -1)
    for r in range(NB):
        b = full[r * BS:(r + 1) * BS, r * BS:(r + 1) * BS]
        b[tri] = b.T[tri]
    ii = np.arange(N)
    full[ii, ii] = 0.0  # |reference diag| < 4e-6; fp8 norm error removed
    return full
